# revision 8
# baseline (speedup 1.0000x reference)
"""FBPINN (16 subnets x width-128 depth-4 tanh MLP, partition-of-unity
windows) on 8 Trainium2 NeuronCores.

Strategy:
 - Host: sort points by x, split into 8 equal chunks (one per core).  Each
   2048-point macro-tile only sees the K=4 subnets with the largest relative
   window mass there (dropped relative window mass < ~7e-4, verified at pack
   time); subnet weights are selected per (core, macro-tile) on the host.
 - Device (SPMD, same NEFF on all 8 cores; per-core data differs):
   feature-major layout ([128 features, points]); per subnet: layer 0 as a
   single ACT tanh with per-partition scale/bias (folds W0, centres, scales,
   b0), 3 hidden layers as fp32r PE matmuls (+ ACT tanh from PSUM), output
   layer as M=32 zero-padded matmuls writing all 4 subnets into one PSUM tile
   at partitions {0,32,64,96}; windows as 2 ACT sigmoids per macro-tile;
   blend on DVE ((raw+bout)*wlo*whi); partition-reduce via an fp32r
   ones-matmul.
 - Host: unpermute the gathered outputs.

The kernel is ACT-bound (~0.70 ns/col f32-out measured): 18 ACT x 2048 cols
per macro-tile ~= 26 us; PE (~15 us f32r full-rate) and DVE (~6 us) hide
underneath.
"""
import os
import sys
from contextlib import ExitStack

for _p in ("/opt/trn_rl_repo",):
    if os.path.isdir(_p) and _p not in sys.path:
        sys.path.insert(0, _p)

import numpy as np
import ml_dtypes

N_PTS = 65536
S = 16           # total subnets
WID = 128        # MLP width
NHID = 3         # hidden->hidden layers (DEPTH-1)
NCORES = 8
NCORE = N_PTS // NCORES          # 8192 points per core
K = 4                            # max subnet slots per macro-tile
TS = 2048                        # macro-tile (points) = 4 psum banks
NMT = NCORE // TS
# slots per mt-index: tiles are permuted across (core, mt) so the 8 tiles
# sharing an mt-index are the ones that tolerate that slot count (the same
# SPMD program runs on every core, so K is baked per mt-index).  mt 0 gets
# the 8 tiles with the largest 4th-subnet window mass.
K_BY_MT = (4, 3, 3, 3)
EPSC = 1e-8

# matmul dtype: "f32r" (default; full PE rate, ~f32 accuracy) |
# "f16"/"bf16" (full rate, 16-bit weights+activations)
MM_DT = os.environ.get("FBPINN_MM_DT", "f32r")

_BUILT = {}


def _build_module(mm_dt, reps=1, hbufs=6):
    import concourse.tile as tile
    from concourse import bacc, mybir

    F32 = mybir.dt.float32
    F32R = mybir.dt.float32r
    MDT = {"bf16": mybir.dt.bfloat16, "f16": mybir.dt.float16,
           "f32r": F32R}.get(mm_dt, F32)
    TANH = mybir.ActivationFunctionType.Tanh
    SIG = mybir.ActivationFunctionType.Sigmoid
    ADD = mybir.AluOpType.add
    MULT = mybir.AluOpType.mult

    nc = bacc.Bacc("TRN2", target_bir_lowering=False, debug=False)

    x_d = nc.dram_tensor("x", [1, NCORE], F32, kind="ExternalInput").ap()
    l0s_d = nc.dram_tensor("l0s", [128, NMT * K], F32, kind="ExternalInput").ap()
    l0b_d = nc.dram_tensor("l0b", [128, NMT * K], F32, kind="ExternalInput").ap()
    whT_d = nc.dram_tensor("whT", [128, NMT * K * NHID * WID], MDT, kind="ExternalInput").ap()
    bhc_d = nc.dram_tensor("bhc", [128, NMT * K * NHID], F32, kind="ExternalInput").ap()
    wout_d = nc.dram_tensor("wout", [128, NMT * K * 128], MDT, kind="ExternalInput").ap()
    boutc_d = nc.dram_tensor("boutc", [128, NMT], F32, kind="ExternalInput").ap()
    wsl_d = nc.dram_tensor("wsl", [128, NMT], F32, kind="ExternalInput").ap()
    wbl_d = nc.dram_tensor("wbl", [128, NMT], F32, kind="ExternalInput").ap()
    wsh_d = nc.dram_tensor("wsh", [128, NMT], F32, kind="ExternalInput").ap()
    wbh_d = nc.dram_tensor("wbh", [128, NMT], F32, kind="ExternalInput").ap()
    ones_d = nc.dram_tensor("ones1", [128, 1], MDT, kind="ExternalInput").ap()
    out_d = nc.dram_tensor("out", [1, NCORE], F32, kind="ExternalOutput").ap()

    # pr (blend output) must be produced in the matmul dtype so the f32r
    # ones-reduce accepts it; for 16-bit modes keep pr in f32 and pay the
    # 4x fp32 sum matmul (matches the old baseline behaviour).
    PRDT = F32R if mm_dt == "f32r" else F32
    SUMDT = MDT if mm_dt == "f32r" else F32

    with tile.TileContext(nc) as tc:
        with ExitStack() as ctx:
            const = ctx.enter_context(tc.tile_pool(name="const", bufs=1))
            xrp = ctx.enter_context(tc.tile_pool(name="xr", bufs=1))
            xbp = ctx.enter_context(tc.tile_pool(name="xb", bufs=3))
            hp = ctx.enter_context(tc.tile_pool(name="h", bufs=hbufs))
            wmp = ctx.enter_context(tc.tile_pool(name="wm", bufs=4))
            prp = ctx.enter_context(tc.tile_pool(name="pr", bufs=2))
            orp = ctx.enter_context(tc.tile_pool(name="or", bufs=4))
            G = ctx.enter_context(tc.tile_pool(name="G", bufs=2, space="PSUM"))

            def load_const(shape, dt, src, tag):
                t = const.tile(shape, dt, tag=tag, name=tag)
                nc.sync.dma_start(t[:], src)
                return t

            l0s = load_const([128, NMT * K], F32, l0s_d, "c_l0s")
            l0b = load_const([128, NMT * K], F32, l0b_d, "c_l0b")
            whT_mts = []
            for _m in range(NMT):
                _w = K * NHID * WID
                t = const.tile([128, _w], MDT, tag=f"c_whT{_m}", name=f"c_whT{_m}")
                nc.sync.dma_start(t[:], whT_d[:, _m * _w:(_m + 1) * _w])
                whT_mts.append(t)
            bhc = load_const([128, NMT * K * NHID], F32, bhc_d, "c_bhc")
            wout_mts = []
            for _m in range(NMT):
                _w = K * 128
                t = const.tile([128, _w], MDT, tag=f"c_wout{_m}", name=f"c_wout{_m}")
                nc.sync.dma_start(t[:], wout_d[:, _m * _w:(_m + 1) * _w])
                wout_mts.append(t)
            boutc = load_const([128, NMT], F32, boutc_d, "c_boutc")
            wsl = load_const([128, NMT], F32, wsl_d, "c_wsl")
            wbl = load_const([128, NMT], F32, wbl_d, "c_wbl")
            wsh = load_const([128, NMT], F32, wsh_d, "c_wsh")
            wbh = load_const([128, NMT], F32, wbh_d, "c_wbh")
            ones1 = load_const([128, 1], SUMDT, ones_d, "c_ones")

            def make_xb(mt):
                sl = slice(mt * TS, (mt + 1) * TS)
                xr = xrp.tile([1, TS], F32, tag="xr", name="xr")
                nc.sync.dma_start(xr[:], x_d[0:1, sl])
                xb = xbp.tile([128, TS], F32, tag="xb", name="xb")
                nc.gpsimd.partition_broadcast(xb[:], xr[0:1, :])
                return xb

            for mt in range(NMT * reps):
                mt = mt % NMT
                sl = slice(mt * TS, (mt + 1) * TS)
                xb = make_xb(mt)

                def emit_l0(k):
                    c = mt * K + k
                    h0 = hp.tile([128, TS], MDT, tag="h", name="h")
                    nc.scalar.activation(h0[:], xb[:], TANH,
                                         bias=l0b[:, c:c + 1],
                                         scale=l0s[:, c:c + 1])
                    return h0

                def emit_hidden(k, l, h_in):
                    g = G.tile([128, TS], F32, tag="G", name="G")
                    whT = whT_mts[mt]
                    off = (k * NHID + l) * WID
                    for s in range(TS // 512):
                        nc.tensor.matmul(
                            g[:, s * 512:(s + 1) * 512],
                            whT[:, off:off + WID],
                            h_in[:, s * 512:(s + 1) * 512],
                            start=True, stop=True)
                    hn = hp.tile([128, TS], MDT, tag="h", name="h")
                    cb = (mt * K + k) * NHID + l
                    nc.scalar.activation(hn[:], g[:], TANH,
                                         bias=bhc[:, cb:cb + 1],
                                         scale=1.0)
                    return hn

                def emit_lout(hs):
                    # fp32r matmuls only support tile_position 0: instead
                    # each slot's lhsT is a zero-padded [128,128] with the
                    # head weights in free-column 32k; the 4 slot matmuls
                    # accumulate into one PSUM tile (raw_k lands on row 32k).
                    go = G.tile([128, TS], F32, tag="G", name="G")
                    wout = wout_mts[mt]
                    Kmt = K_BY_MT[mt]
                    for s in range(TS // 512):
                        for k in range(Kmt):
                            nc.tensor.matmul(
                                go[:, s * 512:(s + 1) * 512],
                                wout[:, k * 128:(k + 1) * 128],
                                hs[k][:, s * 512:(s + 1) * 512],
                                start=(k == 0), stop=(k == Kmt - 1))
                    return go

                def emit_windows():
                    wlo = wmp.tile([128, TS], F32, tag="wlo", name="wlo")
                    nc.scalar.activation(wlo[:], xb[:], SIG,
                                         bias=wbl[:, mt:mt + 1], scale=wsl[:, mt:mt + 1])
                    whi = wmp.tile([128, TS], F32, tag="whi", name="whi")
                    nc.scalar.activation(whi[:], xb[:], SIG,
                                         bias=wbh[:, mt:mt + 1], scale=wsh[:, mt:mt + 1])
                    return wlo, whi

                Kmt = K_BY_MT[mt]
                hs = {k: emit_l0(k) for k in range(Kmt)}
                wlo, whi = emit_windows()
                for l in range(NHID):
                    for k in range(Kmt):
                        hs[k] = emit_hidden(k, l, hs[k])
                go = emit_lout(hs)
                pr = prp.tile([128, TS], PRDT, tag="pr", name="pr")
                nc.vector.scalar_tensor_tensor(pr[:], go[:],
                                               boutc[:, mt:mt + 1],
                                               wlo[:], op0=ADD, op1=MULT)
                nc.vector.tensor_tensor(pr[:], pr[:], whi[:], op=MULT)
                # reduce over partitions: ones-matmul accumulating the
                # blended products into row 0 of the (consumed) psum tile.
                for s in range(TS // 512):
                    nc.tensor.matmul(
                        go[0:1, s * 512:(s + 1) * 512],
                        ones1[:, 0:1],
                        pr[:, s * 512:(s + 1) * 512],
                        start=True, stop=True,
                        tile_position=(0, 0))
                    orow = orp.tile([1, 512], F32, tag="or", name="or")
                    nc.vector.tensor_copy(orow[:],
                                          go[0:1, s * 512:(s + 1) * 512])
                    nc.sync.dma_start(
                        out_d[0:1, mt * TS + s * 512:mt * TS + (s + 1) * 512],
                        orow[:])
    nc.compile()
    return nc


BUILD_OPTS = {}  # extra kwargs for _build_module (variant experiments)


def _get_module(mm_dt, reps=1):
    key = (mm_dt, reps, tuple(sorted(BUILD_OPTS.items())))
    if key not in _BUILT:
        _BUILT[key] = _build_module(mm_dt, reps, **BUILD_OPTS)
    return _BUILT[key]


def _pack_inputs(inputs, mm_dt):
    """Host prep: sort x, route subnets, build per-core in_maps (fp64 math)."""
    x = np.asarray(inputs["x"], dtype=np.float32)            # (N,1)
    W0 = np.asarray(inputs["W0"], dtype=np.float64)          # (S,128,1)
    b0 = np.asarray(inputs["b0"], dtype=np.float64)          # (S,128)
    Wh = np.asarray(inputs["Wh"], dtype=np.float64)          # (S,3,128,128)
    bh = np.asarray(inputs["bh"], dtype=np.float64)          # (S,3,128)
    Wout = np.asarray(inputs["Wout"], dtype=np.float64)      # (S,1,128)
    bout = np.asarray(inputs["bout"], dtype=np.float64)      # (S,1)
    centres = np.asarray(inputs["centres"], dtype=np.float64)[:, 0]
    scales = np.asarray(inputs["scales"], dtype=np.float64)[:, 0]
    mu_min = np.asarray(inputs["mu_min"], dtype=np.float64)[:, 0]
    sd_min = np.asarray(inputs["sd_min"], dtype=np.float64)[:, 0]
    mu_max = np.asarray(inputs["mu_max"], dtype=np.float64)[:, 0]
    sd_max = np.asarray(inputs["sd_max"], dtype=np.float64)[:, 0]

    x0 = x[:, 0]
    order = np.argsort(x0, kind="stable")
    xs = x0[order].astype(np.float64)

    # ---- tile -> (core, mt-class) assignment ----
    # proxy badness of running a tile at K=3: max dropped relative window
    # mass.  The 8 worst tiles form class mt=0 (K=4); the rest run K=3.
    NT = NCORES * NMT
    tiles = xs.reshape(NT, TS)
    badness = np.empty(NT)
    for g in range(NT):
        xc = tiles[g]
        wm = (1.0 / (1.0 + np.exp(-(xc[None, :] - mu_min[:, None]) / sd_min[:, None]))
              * 1.0 / (1.0 + np.exp(-(mu_max[:, None] - xc[None, :]) / sd_max[:, None])))
        rel = wm / wm.sum(0)[None, :]
        top3 = np.argsort(-rel.max(1))[:3]
        badness[g] = np.delete(rel, top3, axis=0).sum(0).max()
    rank = np.argsort(-badness)
    n_k4 = sum(1 for kk in K_BY_MT if kk == 4) * NCORES
    classes = [np.sort(rank[:n_k4].reshape(-1)) if False else None] * NMT
    by_class = []
    off = 0
    for mt in range(NMT):
        cnt = NCORES
        grp = np.sort(rank[off:off + cnt])
        by_class.append(grp)
        off += cnt
    # core c processes tile by_class[mt][c] as its mt-th macro-tile
    tile_of = np.stack(by_class, axis=1)         # (NCORES, NMT) -> global tile
    perm = np.concatenate([tiles[tile_of[c]].reshape(-1) for c in range(NCORES)])
    chunks = perm.reshape(NCORES, NCORE)
    # order: position in `chunks` -> original point index
    tile_order = order.reshape(NT, TS)
    order = np.concatenate([tile_order[tile_of[c]].reshape(-1)
                            for c in range(NCORES)])

    # layer-0 fold: tanh(W0*(x-c)/max(sc,eps) + b0) = tanh(A*x + B)
    scl = np.maximum(scales, EPSC)
    A = W0[:, :, 0] / scl[:, None]                            # (S,128)
    B = b0 - A * centres[:, None]                             # (S,128)

    wdt = {"bf16": ml_dtypes.bfloat16, "f16": np.float16}.get(mm_dt, np.float32)

    in_maps = []
    for c in range(NCORES):
        l0s = np.zeros((128, NMT * K), np.float32)
        l0b = np.zeros((128, NMT * K), np.float32)
        whT = np.zeros((128, NMT * K * NHID * WID), np.float64)
        bhc = np.zeros((128, NMT * K * NHID), np.float32)
        wout = np.zeros((128, NMT * K * 128), np.float64)
        boutc = np.zeros((128, NMT), np.float32)
        wsl = np.zeros((128, NMT), np.float32)
        wbl = np.zeros((128, NMT), np.float32)
        wsh = np.zeros((128, NMT), np.float32)
        wbh = np.zeros((128, NMT), np.float32)
        for mt in range(NMT):
            Kmt = K_BY_MT[mt]
            xc = chunks[c][mt * TS:(mt + 1) * TS]
            wm = (1.0 / (1.0 + np.exp(-(xc[None, :] - mu_min[:, None]) / sd_min[:, None]))
                  * 1.0 / (1.0 + np.exp(-(mu_max[:, None] - xc[None, :]) / sd_max[:, None])))
            tot = wm.sum(0)
            sig = (wm / tot[None, :]).max(1)
            top = np.sort(np.argsort(-sig)[:Kmt])
            dropped = wm[[s for s in range(S) if s not in set(top)]].sum(0) / tot
            if dropped.size and dropped.max() > 4e-2:
                raise RuntimeError(
                    f"routing drop too large on core {c} mt {mt}: {dropped.max():.2e}")
            for kslot, s in enumerate(top):
                row = 32 * kslot
                ck = mt * K + kslot
                l0s[:, ck] = A[s]
                l0b[:, ck] = B[s]
                for l in range(NHID):
                    whT[:, (ck * NHID + l) * WID:(ck * NHID + l + 1) * WID] = Wh[s, l].T
                    bhc[:, ck * NHID + l] = bh[s, l]
                wout[:, ck * 128 + row] = Wout[s, 0]
                boutc[row, mt] = bout[s, 0]
                wsl[row, mt] = 1.0 / sd_min[s]
                wbl[row, mt] = -mu_min[s] / sd_min[s]
                wsh[row, mt] = -1.0 / sd_max[s]
                wbh[row, mt] = mu_max[s] / sd_max[s]
        xc = chunks[c]

        in_maps.append(dict(
            x=np.ascontiguousarray(xc.astype(np.float32)[None, :]),
            ones1=np.ones((128, 1), np.float32 if mm_dt == "f32r" else wdt),
            l0s=l0s, l0b=l0b,
            whT=np.ascontiguousarray(whT.astype(wdt)),
            bhc=bhc,
            wout=np.ascontiguousarray(wout.astype(wdt)),
            boutc=boutc, wsl=wsl, wbl=wbl, wsh=wsh, wbh=wbh,
        ))
    return in_maps, order


def kernel(**inputs) -> np.ndarray:
    import time as _time
    mm_dt = MM_DT
    in_maps, order = _pack_inputs(inputs, mm_dt)
    nc = _get_module(mm_dt)
    from concourse.bass_utils import run_bass_kernel_spmd
    last_err = None
    for attempt in range(3):
        try:
            res = run_bass_kernel_spmd(nc, in_maps, core_ids=list(range(NCORES)))
            break
        except Exception as e:  # transient NRT/axon failures; retry
            last_err = e
            try:
                import jax
                jax.clear_caches()
                jax.extend.backend.clear_backends()
            except Exception:
                pass
            _time.sleep(3.0)
    else:
        raise last_err
    ys = np.concatenate([r["out"][0] for r in res.results])   # sorted order
    out = np.empty(N_PTS, np.float32)
    out[order] = ys
    return out[:, None]


# ---- helpers for test.py (not used by the grading harness) ----

def run_traced(inputs, mm_dt=None, trace_cores=None):
    mm_dt = mm_dt or MM_DT
    in_maps, order = _pack_inputs(inputs, mm_dt)
    nc = _get_module(mm_dt)
    from concourse.bass_utils import run_bass_kernel_spmd
    res = run_bass_kernel_spmd(nc, in_maps, core_ids=list(range(NCORES)),
                               trace=True, trace_cores=trace_cores)
    ys = np.concatenate([r["out"][0] for r in res.results])
    out = np.empty(N_PTS, np.float32)
    out[order] = ys
    return out[:, None], res


def sim_check(inputs, mm_dt=None, cores=(0, 3)):
    """Run CoreSim on a few cores and compare against a numpy reference."""
    mm_dt = mm_dt or MM_DT
    from concourse.bass_interp import CoreSim
    in_maps, order = _pack_inputs(inputs, mm_dt)
    nc = _get_module(mm_dt)
    errs = {}
    for c in cores:
        sim = CoreSim(nc, require_finite=False, require_nnan=False)
        for name, val in in_maps[c].items():
            sim.tensor(name)[:] = val
        sim.simulate()
        got = np.array(sim.tensor("out"))[0]
        exp = _numpy_core_ref(inputs, in_maps[c])
        errs[c] = np.abs(got - exp).max() / max(np.abs(exp).max(), 1e-30)
    return errs


def _numpy_core_ref(inputs, im):
    """fp32 numpy reference for one core's chunk using the packed slots."""
    xall = im["x"][0].astype(np.float32)                     # (NCORE,)
    acc = np.zeros(NCORE, np.float64)
    for mt in range(NMT):
        x = xall[mt * TS:(mt + 1) * TS]
        for kslot in range(K_BY_MT[mt]):
            row = 32 * kslot
            ck = mt * K + kslot
            h = np.tanh(np.float32(im["l0s"][:, ck])[:, None] * x[None, :]
                        + np.float32(im["l0b"][:, ck])[:, None])
            for l in range(NHID):
                Wl = im["whT"][:, (ck * NHID + l) * WID:(ck * NHID + l + 1) * WID].astype(np.float32)
                h = np.tanh(Wl.T @ h + im["bhc"][:, ck * NHID + l].astype(np.float32)[:, None])
            raw = im["wout"][:, ck * 128 + row].astype(np.float32) @ h + im["boutc"][row, mt]
            wlo = 1.0 / (1.0 + np.exp(-(im["wsl"][row, mt] * x + im["wbl"][row, mt])))
            whi = 1.0 / (1.0 + np.exp(-(im["wsh"][row, mt] * x + im["wbh"][row, mt])))
            acc[mt * TS:(mt + 1) * TS] += (raw * wlo * whi).astype(np.float64)
    return acc.astype(np.float32)


# revision 9
# speedup vs baseline: 1.6431x; 1.6431x over previous
"""FBPINN (16 subnets x width-128 depth-4 tanh MLP, partition-of-unity
windows) on 8 Trainium2 NeuronCores.

Strategy:
 - Host: sort points by x, split into 8 equal chunks (one per core).  Each
   2048-point macro-tile only sees the K=4 subnets with the largest relative
   window mass there (dropped relative window mass < ~7e-4, verified at pack
   time); subnet weights are selected per (core, macro-tile) on the host.
 - Device (SPMD, same NEFF on all 8 cores; per-core data differs):
   feature-major layout ([128 features, points]); per subnet: layer 0 as a
   single ACT tanh with per-partition scale/bias (folds W0, centres, scales,
   b0), 3 hidden layers as fp32r PE matmuls (+ ACT tanh from PSUM), output
   layer as M=32 zero-padded matmuls writing all 4 subnets into one PSUM tile
   at partitions {0,32,64,96}; windows as 2 ACT sigmoids per macro-tile;
   blend on DVE ((raw+bout)*wlo*whi); partition-reduce via an fp32r
   ones-matmul.
 - Host: unpermute the gathered outputs.

The kernel is ACT-bound (~0.70 ns/col f32-out measured): 18 ACT x 2048 cols
per macro-tile ~= 26 us; PE (~15 us f32r full-rate) and DVE (~6 us) hide
underneath.
"""
import os
import sys
from contextlib import ExitStack

for _p in ("/opt/trn_rl_repo",):
    if os.path.isdir(_p) and _p not in sys.path:
        sys.path.insert(0, _p)

import numpy as np
import ml_dtypes

N_PTS = 65536
S = 16           # total subnets
WID = 128        # MLP width
NHID = 3         # hidden->hidden layers (DEPTH-1)
NCORES = 8
NCORE = N_PTS // NCORES          # 8192 points per core
K = 4                            # max subnet slots per macro-tile
TS = 2048                        # macro-tile (points) = 4 psum banks
NMT = NCORE // TS
# slots per mt-index: tiles are permuted across (core, mt) so the 8 tiles
# sharing an mt-index are the ones that tolerate that slot count (the same
# SPMD program runs on every core, so K is baked per mt-index).  mt 0 gets
# the 8 tiles with the largest 4th-subnet window mass.
K_BY_MT = (4, 3, 3, 3)
EPSC = 1e-8

# matmul dtype: "f32r" (default; full PE rate, ~f32 accuracy) |
# "f16"/"bf16" (full rate, 16-bit weights+activations)
MM_DT = os.environ.get("FBPINN_MM_DT", "f32r")

_BUILT = {}


def _build_module(mm_dt, reps=1, hbufs=6, loop=1):
    import concourse.tile as tile
    from concourse import bacc, mybir

    F32 = mybir.dt.float32
    F32R = mybir.dt.float32r
    MDT = {"bf16": mybir.dt.bfloat16, "f16": mybir.dt.float16,
           "f32r": F32R}.get(mm_dt, F32)
    TANH = mybir.ActivationFunctionType.Tanh
    SIG = mybir.ActivationFunctionType.Sigmoid
    ADD = mybir.AluOpType.add
    MULT = mybir.AluOpType.mult

    nc = bacc.Bacc("TRN2", target_bir_lowering=False, debug=False)

    x_d = nc.dram_tensor("x", [1, NCORE], F32, kind="ExternalInput").ap()
    l0s_d = nc.dram_tensor("l0s", [128, NMT * K], F32, kind="ExternalInput").ap()
    l0b_d = nc.dram_tensor("l0b", [128, NMT * K], F32, kind="ExternalInput").ap()
    whT_d = nc.dram_tensor("whT", [128, NMT * K * NHID * WID], MDT, kind="ExternalInput").ap()
    bhc_d = nc.dram_tensor("bhc", [128, NMT * K * NHID], F32, kind="ExternalInput").ap()
    wout_d = nc.dram_tensor("wout", [128, NMT * K * 128], MDT, kind="ExternalInput").ap()
    boutc_d = nc.dram_tensor("boutc", [128, NMT], F32, kind="ExternalInput").ap()
    wsl_d = nc.dram_tensor("wsl", [128, NMT], F32, kind="ExternalInput").ap()
    wbl_d = nc.dram_tensor("wbl", [128, NMT], F32, kind="ExternalInput").ap()
    wsh_d = nc.dram_tensor("wsh", [128, NMT], F32, kind="ExternalInput").ap()
    wbh_d = nc.dram_tensor("wbh", [128, NMT], F32, kind="ExternalInput").ap()
    ones_d = nc.dram_tensor("ones1", [128, 1], MDT, kind="ExternalInput").ap()
    out_d = nc.dram_tensor("out", [1, NCORE], F32, kind="ExternalOutput").ap()

    # pr (blend output) must be produced in the matmul dtype so the f32r
    # ones-reduce accepts it; for 16-bit modes keep pr in f32 and pay the
    # 4x fp32 sum matmul (matches the old baseline behaviour).
    PRDT = F32R if mm_dt == "f32r" else F32
    SUMDT = MDT if mm_dt == "f32r" else F32

    with tile.TileContext(nc) as tc:
        with ExitStack() as ctx:
            const = ctx.enter_context(tc.tile_pool(name="const", bufs=1))
            xrp = ctx.enter_context(tc.tile_pool(name="xr", bufs=1))
            xbp = ctx.enter_context(tc.tile_pool(name="xb", bufs=3))
            hp = ctx.enter_context(tc.tile_pool(name="h", bufs=hbufs))
            wmp = ctx.enter_context(tc.tile_pool(name="wm", bufs=4))
            prp = ctx.enter_context(tc.tile_pool(name="pr", bufs=2))
            orp = ctx.enter_context(tc.tile_pool(name="or", bufs=4))
            G = ctx.enter_context(tc.tile_pool(name="G", bufs=2, space="PSUM"))

            def load_const(shape, dt, src, tag):
                t = const.tile(shape, dt, tag=tag, name=tag)
                nc.sync.dma_start(t[:], src)
                return t

            l0s = load_const([128, NMT * K], F32, l0s_d, "c_l0s")
            l0b = load_const([128, NMT * K], F32, l0b_d, "c_l0b")
            whT_mts = []
            for _m in range(NMT):
                _w = K * NHID * WID
                t = const.tile([128, _w], MDT, tag=f"c_whT{_m}", name=f"c_whT{_m}")
                nc.sync.dma_start(t[:], whT_d[:, _m * _w:(_m + 1) * _w])
                whT_mts.append(t)
            bhc = load_const([128, NMT * K * NHID], F32, bhc_d, "c_bhc")
            wout_mts = []
            for _m in range(NMT):
                _w = K * 128
                t = const.tile([128, _w], MDT, tag=f"c_wout{_m}", name=f"c_wout{_m}")
                nc.sync.dma_start(t[:], wout_d[:, _m * _w:(_m + 1) * _w])
                wout_mts.append(t)
            boutc = load_const([128, NMT], F32, boutc_d, "c_boutc")
            wsl = load_const([128, NMT], F32, wsl_d, "c_wsl")
            wbl = load_const([128, NMT], F32, wbl_d, "c_wbl")
            wsh = load_const([128, NMT], F32, wsh_d, "c_wsh")
            wbh = load_const([128, NMT], F32, wbh_d, "c_wbh")
            ones1 = load_const([128, 1], SUMDT, ones_d, "c_ones")

            def make_xb(mt):
                sl = slice(mt * TS, (mt + 1) * TS)
                xr = xrp.tile([1, TS], F32, tag="xr", name="xr")
                nc.sync.dma_start(xr[:], x_d[0:1, sl])
                xb = xbp.tile([128, TS], F32, tag="xb", name="xb")
                nc.gpsimd.partition_broadcast(xb[:], xr[0:1, :])
                return xb

            def emit_body():
              for mt in range(NMT * reps):
                mt = mt % NMT
                sl = slice(mt * TS, (mt + 1) * TS)
                xb = make_xb(mt)

                def emit_l0(k):
                    c = mt * K + k
                    h0 = hp.tile([128, TS], MDT, tag="h", name="h")
                    nc.scalar.activation(h0[:], xb[:], TANH,
                                         bias=l0b[:, c:c + 1],
                                         scale=l0s[:, c:c + 1])
                    return h0

                def emit_hidden(k, l, h_in):
                    g = G.tile([128, TS], F32, tag="G", name="G")
                    whT = whT_mts[mt]
                    off = (k * NHID + l) * WID
                    for s in range(TS // 512):
                        nc.tensor.matmul(
                            g[:, s * 512:(s + 1) * 512],
                            whT[:, off:off + WID],
                            h_in[:, s * 512:(s + 1) * 512],
                            start=True, stop=True)
                    hn = hp.tile([128, TS], MDT, tag="h", name="h")
                    cb = (mt * K + k) * NHID + l
                    nc.scalar.activation(hn[:], g[:], TANH,
                                         bias=bhc[:, cb:cb + 1],
                                         scale=1.0)
                    return hn

                def emit_lout(hs):
                    # fp32r matmuls only support tile_position 0: instead
                    # each slot's lhsT is a zero-padded [128,128] with the
                    # head weights in free-column 32k; the 4 slot matmuls
                    # accumulate into one PSUM tile (raw_k lands on row 32k).
                    go = G.tile([128, TS], F32, tag="G", name="G")
                    wout = wout_mts[mt]
                    Kmt = K_BY_MT[mt]
                    for s in range(TS // 512):
                        for k in range(Kmt):
                            nc.tensor.matmul(
                                go[:, s * 512:(s + 1) * 512],
                                wout[:, k * 128:(k + 1) * 128],
                                hs[k][:, s * 512:(s + 1) * 512],
                                start=(k == 0), stop=(k == Kmt - 1))
                    return go

                def emit_windows():
                    wlo = wmp.tile([128, TS], F32, tag="wlo", name="wlo")
                    nc.scalar.activation(wlo[:], xb[:], SIG,
                                         bias=wbl[:, mt:mt + 1], scale=wsl[:, mt:mt + 1])
                    whi = wmp.tile([128, TS], F32, tag="whi", name="whi")
                    nc.scalar.activation(whi[:], xb[:], SIG,
                                         bias=wbh[:, mt:mt + 1], scale=wsh[:, mt:mt + 1])
                    return wlo, whi

                Kmt = K_BY_MT[mt]
                hs = {k: emit_l0(k) for k in range(Kmt)}
                wlo, whi = emit_windows()
                for l in range(NHID):
                    for k in range(Kmt):
                        hs[k] = emit_hidden(k, l, hs[k])
                go = emit_lout(hs)
                pr = prp.tile([128, TS], PRDT, tag="pr", name="pr")
                nc.vector.scalar_tensor_tensor(pr[:], go[:],
                                               boutc[:, mt:mt + 1],
                                               wlo[:], op0=ADD, op1=MULT)
                nc.vector.tensor_tensor(pr[:], pr[:], whi[:], op=MULT)
                # reduce over partitions: ones-matmul accumulating the
                # blended products into row 0 of the (consumed) psum tile.
                for s in range(TS // 512):
                    nc.tensor.matmul(
                        go[0:1, s * 512:(s + 1) * 512],
                        ones1[:, 0:1],
                        pr[:, s * 512:(s + 1) * 512],
                        start=True, stop=True,
                        tile_position=(0, 0))
                    orow = orp.tile([1, 512], F32, tag="or", name="or")
                    nc.vector.tensor_copy(orow[:],
                                          go[0:1, s * 512:(s + 1) * 512])
                    nc.sync.dma_start(
                        out_d[0:1, mt * TS + s * 512:mt * TS + (s + 1) * 512],
                        orow[:])
            if loop > 1:
                with tc.For_i(0, loop, 1):
                    emit_body()
            else:
                emit_body()
    nc.compile()
    return nc


BUILD_OPTS = {}  # extra kwargs for _build_module (variant experiments)


def _get_module(mm_dt, reps=1, loop=1):
    key = (mm_dt, reps, loop, tuple(sorted(BUILD_OPTS.items())))
    if key not in _BUILT:
        _BUILT[key] = _build_module(mm_dt, reps, loop=loop, **BUILD_OPTS)
    return _BUILT[key]


def _pack_inputs(inputs, mm_dt):
    """Host prep: sort x, route subnets, build per-core in_maps (fp64 math)."""
    x = np.asarray(inputs["x"], dtype=np.float32)            # (N,1)
    W0 = np.asarray(inputs["W0"], dtype=np.float64)          # (S,128,1)
    b0 = np.asarray(inputs["b0"], dtype=np.float64)          # (S,128)
    Wh = np.asarray(inputs["Wh"], dtype=np.float64)          # (S,3,128,128)
    bh = np.asarray(inputs["bh"], dtype=np.float64)          # (S,3,128)
    Wout = np.asarray(inputs["Wout"], dtype=np.float64)      # (S,1,128)
    bout = np.asarray(inputs["bout"], dtype=np.float64)      # (S,1)
    centres = np.asarray(inputs["centres"], dtype=np.float64)[:, 0]
    scales = np.asarray(inputs["scales"], dtype=np.float64)[:, 0]
    mu_min = np.asarray(inputs["mu_min"], dtype=np.float64)[:, 0]
    sd_min = np.asarray(inputs["sd_min"], dtype=np.float64)[:, 0]
    mu_max = np.asarray(inputs["mu_max"], dtype=np.float64)[:, 0]
    sd_max = np.asarray(inputs["sd_max"], dtype=np.float64)[:, 0]

    x0 = x[:, 0]
    order = np.argsort(x0, kind="stable")
    xs = x0[order].astype(np.float64)

    # ---- tile -> (core, mt-class) assignment ----
    # proxy badness of running a tile at K=3: max dropped relative window
    # mass.  The 8 worst tiles form class mt=0 (K=4); the rest run K=3.
    NT = NCORES * NMT
    tiles = xs.reshape(NT, TS)
    badness = np.empty(NT)
    for g in range(NT):
        xc = tiles[g]
        wm = (1.0 / (1.0 + np.exp(-(xc[None, :] - mu_min[:, None]) / sd_min[:, None]))
              * 1.0 / (1.0 + np.exp(-(mu_max[:, None] - xc[None, :]) / sd_max[:, None])))
        rel = wm / wm.sum(0)[None, :]
        top3 = np.argsort(-rel.max(1))[:3]
        badness[g] = np.delete(rel, top3, axis=0).sum(0).max()
    rank = np.argsort(-badness)
    n_k4 = sum(1 for kk in K_BY_MT if kk == 4) * NCORES
    classes = [np.sort(rank[:n_k4].reshape(-1)) if False else None] * NMT
    by_class = []
    off = 0
    for mt in range(NMT):
        cnt = NCORES
        grp = np.sort(rank[off:off + cnt])
        by_class.append(grp)
        off += cnt
    # core c processes tile by_class[mt][c] as its mt-th macro-tile
    tile_of = np.stack(by_class, axis=1)         # (NCORES, NMT) -> global tile
    perm = np.concatenate([tiles[tile_of[c]].reshape(-1) for c in range(NCORES)])
    chunks = perm.reshape(NCORES, NCORE)
    # order: position in `chunks` -> original point index
    tile_order = order.reshape(NT, TS)
    order = np.concatenate([tile_order[tile_of[c]].reshape(-1)
                            for c in range(NCORES)])

    # layer-0 fold: tanh(W0*(x-c)/max(sc,eps) + b0) = tanh(A*x + B)
    scl = np.maximum(scales, EPSC)
    A = W0[:, :, 0] / scl[:, None]                            # (S,128)
    B = b0 - A * centres[:, None]                             # (S,128)

    wdt = {"bf16": ml_dtypes.bfloat16, "f16": np.float16}.get(mm_dt, np.float32)

    in_maps = []
    for c in range(NCORES):
        l0s = np.zeros((128, NMT * K), np.float32)
        l0b = np.zeros((128, NMT * K), np.float32)
        whT = np.zeros((128, NMT * K * NHID * WID), np.float64)
        bhc = np.zeros((128, NMT * K * NHID), np.float32)
        wout = np.zeros((128, NMT * K * 128), np.float64)
        boutc = np.zeros((128, NMT), np.float32)
        wsl = np.zeros((128, NMT), np.float32)
        wbl = np.zeros((128, NMT), np.float32)
        wsh = np.zeros((128, NMT), np.float32)
        wbh = np.zeros((128, NMT), np.float32)
        for mt in range(NMT):
            Kmt = K_BY_MT[mt]
            xc = chunks[c][mt * TS:(mt + 1) * TS]
            wm = (1.0 / (1.0 + np.exp(-(xc[None, :] - mu_min[:, None]) / sd_min[:, None]))
                  * 1.0 / (1.0 + np.exp(-(mu_max[:, None] - xc[None, :]) / sd_max[:, None])))
            tot = wm.sum(0)
            sig = (wm / tot[None, :]).max(1)
            top = np.sort(np.argsort(-sig)[:Kmt])
            dropped = wm[[s for s in range(S) if s not in set(top)]].sum(0) / tot
            if dropped.size and dropped.max() > 4e-2:
                raise RuntimeError(
                    f"routing drop too large on core {c} mt {mt}: {dropped.max():.2e}")
            for kslot, s in enumerate(top):
                row = 32 * kslot
                ck = mt * K + kslot
                l0s[:, ck] = A[s]
                l0b[:, ck] = B[s]
                for l in range(NHID):
                    whT[:, (ck * NHID + l) * WID:(ck * NHID + l + 1) * WID] = Wh[s, l].T
                    bhc[:, ck * NHID + l] = bh[s, l]
                wout[:, ck * 128 + row] = Wout[s, 0]
                boutc[row, mt] = bout[s, 0]
                wsl[row, mt] = 1.0 / sd_min[s]
                wbl[row, mt] = -mu_min[s] / sd_min[s]
                wsh[row, mt] = -1.0 / sd_max[s]
                wbh[row, mt] = mu_max[s] / sd_max[s]
        xc = chunks[c]

        in_maps.append(dict(
            x=np.ascontiguousarray(xc.astype(np.float32)[None, :]),
            ones1=np.ones((128, 1), np.float32 if mm_dt == "f32r" else wdt),
            l0s=l0s, l0b=l0b,
            whT=np.ascontiguousarray(whT.astype(wdt)),
            bhc=bhc,
            wout=np.ascontiguousarray(wout.astype(wdt)),
            boutc=boutc, wsl=wsl, wbl=wbl, wsh=wsh, wbh=wbh,
        ))
    return in_maps, order


def kernel(**inputs) -> np.ndarray:
    import time as _time
    mm_dt = MM_DT
    in_maps, order = _pack_inputs(inputs, mm_dt)
    nc = _get_module(mm_dt)
    from concourse.bass_utils import run_bass_kernel_spmd
    last_err = None
    for attempt in range(3):
        try:
            res = run_bass_kernel_spmd(nc, in_maps, core_ids=list(range(NCORES)))
            break
        except Exception as e:  # transient NRT/axon failures; retry
            last_err = e
            try:
                import jax
                jax.clear_caches()
                jax.extend.backend.clear_backends()
            except Exception:
                pass
            _time.sleep(3.0)
    else:
        raise last_err
    ys = np.concatenate([r["out"][0] for r in res.results])   # sorted order
    out = np.empty(N_PTS, np.float32)
    out[order] = ys
    return out[:, None]


# ---- helpers for test.py (not used by the grading harness) ----

def run_traced(inputs, mm_dt=None, trace_cores=None):
    mm_dt = mm_dt or MM_DT
    in_maps, order = _pack_inputs(inputs, mm_dt)
    nc = _get_module(mm_dt)
    from concourse.bass_utils import run_bass_kernel_spmd
    res = run_bass_kernel_spmd(nc, in_maps, core_ids=list(range(NCORES)),
                               trace=True, trace_cores=trace_cores)
    ys = np.concatenate([r["out"][0] for r in res.results])
    out = np.empty(N_PTS, np.float32)
    out[order] = ys
    return out[:, None], res


def sim_check(inputs, mm_dt=None, cores=(0, 3)):
    """Run CoreSim on a few cores and compare against a numpy reference."""
    mm_dt = mm_dt or MM_DT
    from concourse.bass_interp import CoreSim
    in_maps, order = _pack_inputs(inputs, mm_dt)
    nc = _get_module(mm_dt)
    errs = {}
    for c in cores:
        sim = CoreSim(nc, require_finite=False, require_nnan=False)
        for name, val in in_maps[c].items():
            sim.tensor(name)[:] = val
        sim.simulate()
        got = np.array(sim.tensor("out"))[0]
        exp = _numpy_core_ref(inputs, in_maps[c])
        errs[c] = np.abs(got - exp).max() / max(np.abs(exp).max(), 1e-30)
    return errs


def _numpy_core_ref(inputs, im):
    """fp32 numpy reference for one core's chunk using the packed slots."""
    xall = im["x"][0].astype(np.float32)                     # (NCORE,)
    acc = np.zeros(NCORE, np.float64)
    for mt in range(NMT):
        x = xall[mt * TS:(mt + 1) * TS]
        for kslot in range(K_BY_MT[mt]):
            row = 32 * kslot
            ck = mt * K + kslot
            h = np.tanh(np.float32(im["l0s"][:, ck])[:, None] * x[None, :]
                        + np.float32(im["l0b"][:, ck])[:, None])
            for l in range(NHID):
                Wl = im["whT"][:, (ck * NHID + l) * WID:(ck * NHID + l + 1) * WID].astype(np.float32)
                h = np.tanh(Wl.T @ h + im["bhc"][:, ck * NHID + l].astype(np.float32)[:, None])
            raw = im["wout"][:, ck * 128 + row].astype(np.float32) @ h + im["boutc"][row, mt]
            wlo = 1.0 / (1.0 + np.exp(-(im["wsl"][row, mt] * x + im["wbl"][row, mt])))
            whi = 1.0 / (1.0 + np.exp(-(im["wsh"][row, mt] * x + im["wbh"][row, mt])))
            acc[mt * TS:(mt + 1) * TS] += (raw * wlo * whi).astype(np.float64)
    return acc.astype(np.float32)
